# revision 25
# baseline (speedup 1.0000x reference)
"""Self-contained 8-core Trainium2 Bass kernel for nn_MultiHeadAttention.

Problem: x:[4,2048,1024] f32, w_qkv:[3072,1024], b_qkv:[3072],
w_proj:[1024,1024], b_proj:[1024].  16 heads, head_dim 64.

Sharding: core c = batch(4) x head-group(2).  Each core computes QKV for
its 8 heads on its batch, attention, and a partial output projection over
its 512 head-dims.  Host sums the two partials per batch and adds b_proj.

Per-core dataflow (all matmuls bf16, fp32 PSUM):
  - host supplies xT_aug [1152,2048] = [x_b^T; ones; 0pad] (contraction dim
    carries the bias via an augmented row), wqk [1152,1024] with column
    layout head h -> cols h*128..h*128+64 = q (pre-scaled 1/8), +64..+128 = k,
    wv [1152,512], wp [512,1024].
  - prologue: v tiles (natural [n, head, d] layout), then qk for head-pair 0;
    qk for pairs 1-3 interleave into attention PE slack as fillers.
  - qkT[h] tiles [128,2048]: partitions 0:64 = q^T of even head, 64:128 =
    odd head (d on partitions) -> energy^T = k @ q^T lands in [k,q] layout,
    so the exp output feeds att@V directly (no PE transposes anywhere).
  - attention runs per head-PAIR with PE array tiling:
      * energies: the two K=64 matmuls row-tile (array rows 0:64 / 64:128)
        and run concurrently;
      * att@V: col-tiled, head h -> psum partitions 0:64, h' -> 64:128,
        two moving streams into different column groups concurrently;
      * softmax denominators: 4 P=1 col-tiled matmuls (ones vector lhsT)
        accumulating into one den psum bank at partitions 0/32/64/96.
    Per 2.23us pair-slot (2 exps) the PE does ~1.7us of work, so the
    scalar engine's exp stream (256 x [128,1024] exp, ~1.1us each) is the
    pacing floor; emission order exp(kt-1), E(kt), attv(kt-2) keeps it fed,
    and each segment's last two attvs carry into the next segment's first
    two slots so the stream continues across boundaries.
  - energies are bounded (~|2|) so exp needs no max subtraction.
  - denominators: den psum -> sbuf -> DRAM bounce -> packed [32,512] ->
    lane-parallel DVE reciprocal -> stream_shuffle broadcast -> one
    scalar_tensor_tensor multiply normalizes out^T (interleaved as dve
    fillers into the next pair's slots).
  - y partials returned in bf16 (halves the output DMA); host sums in f32.
"""
import sys

sys.path.insert(0, "/opt/trn_rl_repo")

import numpy as np
import ml_dtypes

import concourse.bass as bass
import concourse.mybir as mybir
import concourse.tile as tile
from concourse import bacc
from concourse.bass_utils import run_bass_kernel_spmd

bf16 = ml_dtypes.bfloat16
F32 = mybir.dt.float32
BF16 = mybir.dt.bfloat16

B, N, EMB = 4, 2048, 1024
HEADS, HD = 16, 64
HPC = 8            # heads per core
KAUG = 1152        # 1024 emb + 1 bias row, padded to 9*128
NKT = KAUG // 128  # 9 contraction tiles for qkv
NT_N = N // 128    # 16 n-tiles
EXPF = mybir.ActivationFunctionType.Exp


def _build_kernel(ctx, tc, nc, xT, wqk, wv, wp, y, dbg=None):
    mult = mybir.AluOpType.mult

    const = ctx.enter_context(tc.tile_pool(name="const", bufs=1))
    qkp = ctx.enter_context(tc.tile_pool(name="qkp", bufs=1))
    vp = ctx.enter_context(tc.tile_pool(name="vp", bufs=1))
    outp = ctx.enter_context(tc.tile_pool(name="outp", bufs=1))
    attp = ctx.enter_context(tc.tile_pool(name="attp", bufs=7))
    misc = ctx.enter_context(tc.tile_pool(name="misc", bufs=1))
    stage = ctx.enter_context(tc.tile_pool(name="stage", bufs=2))
    bcp = ctx.enter_context(tc.tile_pool(name="bcp", bufs=3))
    yp = ctx.enter_context(tc.tile_pool(name="yp", bufs=4))
    pe = ctx.enter_context(tc.tile_pool(name="pe", bufs=2, space="PSUM"))
    fill = ctx.enter_context(tc.tile_pool(name="fill", bufs=1, space="PSUM"))
    po = ctx.enter_context(tc.tile_pool(name="po", bufs=2, space="PSUM"))
    dn = ctx.enter_context(tc.tile_pool(name="dn", bufs=1, space="PSUM"))
    dramp = ctx.enter_context(tc.tile_pool(name="dramp", bufs=1, space="DRAM"))

    # ---- load inputs: xT+wv first (v-phase deps), wqk next, wp last;
    # spread across three DMA queues so the prologue isn't one-queue bound ----
    xT_t = []
    wqk_t = []
    wv_t = []
    for kt in range(NKT):
        t = const.tile([128, N], BF16, tag=f"xT{kt}", name=f"xT{kt}")
        (nc.sync if kt % 2 == 0 else nc.scalar).dma_start(
            t[:], xT[kt * 128:(kt + 1) * 128, :])
        xT_t.append(t)
        t = const.tile([128, 512], BF16, tag=f"wv{kt}", name=f"wv{kt}")
        nc.gpsimd.dma_start(t[:], wv[kt * 128:(kt + 1) * 128, :])
        wv_t.append(t)
    for kt in range(NKT):
        t = const.tile([128, 1024], BF16, tag=f"wqk{kt}", name=f"wqk{kt}")
        nc.gpsimd.dma_start(t[:], wqk[kt * 128:(kt + 1) * 128, :])
        wqk_t.append(t)
    wp_t = []
    for t4 in range(4):
        t = const.tile([128, 1024], BF16, tag=f"wp{t4}", name=f"wp{t4}")
        nc.sync.dma_start(t[:], wp[t4 * 128:(t4 + 1) * 128, :])
        wp_t.append(t)

    # qk bias vectors: aug row 1024 of wqk, one [128,1] per m-tile
    bq_t = []
    for t8 in range(8):
        tb = misc.tile([128, 1], BF16, tag=f"bqb{t8}", name=f"bqb{t8}")
        nc.scalar.dma_start(tb[:], wqk[EMB:EMB + 1, t8 * 128:(t8 + 1) * 128])
        t = misc.tile([128, 1], F32, tag=f"bq{t8}", name=f"bq{t8}")
        nc.vector.tensor_copy(t[:], tb[:])
        bq_t.append(t)

    # qk m-tiles 0..3 = q of head pairs (h%2 on partition halves), 4..7 = k.
    qkT = [qkp.tile([128, N], BF16, tag=f"qkT{h}", name=f"qkT{h}") for h in range(HPC)]
    v_t = [vp.tile([128, HPC, 65], BF16, tag=f"v{nt}", name=f"v{nt}") for nt in range(NT_N)]
    outT_raw = [outp.tile([128, N], BF16, tag=f"or{t}", name=f"or{t}") for t in range(4)]
    outT_n = [outp.tile([128, N], BF16, tag=f"on{t}", name=f"on{t}") for t in range(4)]
    den_dram = dramp.tile([32, 512], F32, name="den_dram")
    den_pk = misc.tile([32, 512], F32, tag="den", name="den_pk")
    rec_pk = misc.tile([32, 512], F32, tag="rec", name="rec_pk")
    nc.vector.memset(den_pk[:], 1.0)
    ones_t = misc.tile([128, 1], BF16, tag="ones", name="ones_t")
    nc.vector.memset(ones_t[:], 1.0)

    def emit_v_tile(nt):
        p = pe.tile([128, 1024], F32, tag="pe", name="pep")
        for kt in range(NKT):
            nc.tensor.matmul(
                p[:, 0:512],
                xT_t[kt][:, nt * 128:(nt + 1) * 128],
                wv_t[kt][:],
                start=(kt == 0), stop=(kt == NKT - 1),
            )
        nc.scalar.copy(
            v_t[nt][:, :, 0:64],
            p[:, 0:512].rearrange("p (h c) -> p h c", c=64),
        )
        nc.vector.memset(v_t[nt][:, :, 64:65], 1.0)

    def qk_pair_gen(hp, prologue=False):
        """Generator emitting one PE matmul per step for q/k m-tiles of pair hp.

        prologue=True (pair 0): full [128,1024] psum groups from the pe pool
        (double-buffered, fastest serial path).  Fillers use a single-bank
        [128,512] psum so po can triple-buffer: each j-half gets its own
        accumulation group and bias-add."""
        for t in (hp, 4 + hp):
            for nbp in range(2):
                if prologue:
                    p = pe.tile([128, 1024], F32, tag="pe", name="pep")
                    for j in range(2):
                        pj = p[:, j * 512:(j + 1) * 512]
                        for kt in range(NKT - 1):
                            nc.tensor.matmul(
                                pj,
                                wqk_t[kt][:, t * 128:(t + 1) * 128],
                                xT_t[kt][:, nbp * 1024 + j * 512:
                                         nbp * 1024 + (j + 1) * 512],
                                start=(kt == 0), stop=(kt == NKT - 2),
                            )
                            if not (kt == NKT - 2 and j == 1):
                                yield
                    nc.vector.tensor_scalar_add(
                        qkT[t][:, nbp * 1024:(nbp + 1) * 1024], p[:], bq_t[t][:])
                    yield
                else:
                    for j in range(2):
                        p = fill.tile([128, 512], F32, tag="fl", name="flp")
                        c = nbp * 1024 + j * 512
                        for kt in range(NKT - 1):
                            nc.tensor.matmul(
                                p[:],
                                wqk_t[kt][:, t * 128:(t + 1) * 128],
                                xT_t[kt][:, c:c + 512],
                                start=(kt == 0), stop=(kt == NKT - 2),
                            )
                            yield
                        nc.vector.tensor_scalar_add(
                            qkT[t][:, c:c + 512], p[:], bq_t[t][:])
                        yield
                        yield
                        yield

    def norm_pair(p_, qg):
        # both heads of the pair in one STT: bc rows 0:64 = 1/den of head
        # 2p, 64:128 = head 2p+1 (partition-parallel, so the [128,512] STT
        # costs the same as a [64,512] one)
        bc = bcp.tile([128, 512], F32, tag="bc", name="bc")
        m0 = [(2 * p_) * 4 + qg] * 32
        m1 = [(2 * p_ + 1) * 4 + qg] * 32
        nc.vector.stream_shuffle(bc[0:32, :], rec_pk[0:32, :], mask=m0)
        nc.vector.stream_shuffle(bc[32:64, :], rec_pk[0:32, :], mask=m0)
        nc.vector.stream_shuffle(bc[64:96, :], rec_pk[0:32, :], mask=m1)
        nc.vector.stream_shuffle(bc[96:128, :], rec_pk[0:32, :], mask=m1)
        nc.vector.scalar_tensor_tensor(
            outT_n[p_][:, qg * 512:(qg + 1) * 512],
            outT_raw[p_][:, qg * 512:(qg + 1) * 512],
            1.0,
            bc[:, :],
            op0=mult, op1=mult,
        )

    def v_tail_gen(nts):
        # last V tiles, emitted as pair-0 fillers: psum from the fill pool
        # (free until the qk filler for pair 1 starts behind us in the same
        # chain) and the unpack copy on DVE, not ACT (ACT is the pacer).
        for nt in nts:
            p = fill.tile([128, 512], F32, tag="fl", name="flp")
            for kt in range(NKT):
                nc.tensor.matmul(
                    p[:],
                    xT_t[kt][:, nt * 128:(nt + 1) * 128],
                    wv_t[kt][:],
                    start=(kt == 0), stop=(kt == NKT - 1),
                )
                yield
            nc.vector.tensor_copy(
                v_t[nt][:, :, 0:64],
                p[:].rearrange("p (h c) -> p h c", c=64),
            )
            nc.vector.memset(v_t[nt][:, :, 64:65], 1.0)
            yield

    def chain2(a, b):
        yield from a
        yield from b

    def norm_gen_half(p_, qh):
        # qh==0: normalize q-groups 0,1 of pair p (their outT_raw/den rows
        # drained at mid-pair); qh==1: q-groups 2,3 (drained at pair end).
        # The den reload+reciprocal is split into two half-width steps so no
        # single DVE op exceeds ~1.7us -- keeps the filler bias-adds (which
        # the single-buffered fill psum WARs on) from queueing behind it.
        nrows = 8 * p_ + 6 if qh == 0 else 8 * (p_ + 1)
        nc.gpsimd.dma_start(den_pk[0:nrows, :], den_dram[0:nrows, :])
        nc.vector.reciprocal(rec_pk[:, 0:256], den_pk[:, 0:256])
        yield
        nc.vector.reciprocal(rec_pk[:, 256:512], den_pk[:, 256:512])
        yield
        for qg in (2 * qh, 2 * qh + 1):
            norm_pair(p_, qg)
            yield
            yield

    def den_recip(nrows):
        nc.gpsimd.dma_start(den_pk[0:nrows, :], den_dram[0:nrows, :])
        nc.vector.reciprocal(rec_pk[:], den_pk[:])

    # ---- prologue: v tiles, then qk for head-pair 0 (PE-only, ACT idle) ----
    for nt in range(NT_N - 4):
        emit_v_tile(nt)
    for _ in qk_pair_gen(0, prologue=True):
        pass

    # ---- attention.  The scalar engine's exp stream (one [128,1024] exp per
    # kt, ~1us each, 256 total) is the throughput floor.  Per-slot emission
    # order is exp(kt-1), energy(kt), attv(kt-2): the next energy issues on
    # the PE right behind the previous exp so ACT never waits.  The two
    # trailing attvs of each (h,qh) segment carry into the next segment's
    # first two slots so the exp stream continues across boundaries.  qk
    # matmuls for the next head pair interleave as fillers into PE slack;
    # each pair's generator MUST be fully drained before the pair's energies
    # are emitted (the drain at each even h) -- the energies read qkT tiles
    # that the filler writes. ----
    filler = iter(())
    state = {"dve": iter(())}

    def seg_ctx(p_, qh):
        return {"p": p_, "qh": qh, "e_q": [[], []], "a_q": [[], []],
                "o": None, "dn": None}

    def emit_E(sc, kt):
        # energies for both heads of the pair, row-tiled (h: array rows 0:64,
        # h': rows 64:128) so the two K=64 matmuls run concurrently.
        for hl in range(2):
            pb = hl * 64
            p = pe.tile([128, 1024], F32, tag="pe", name="pep")
            for j in range(2):
                nc.tensor.matmul(
                    p[:, j * 512:(j + 1) * 512],
                    qkT[4 + sc["p"]][pb:pb + 64, kt * 128:(kt + 1) * 128],
                    qkT[sc["p"]][pb:pb + 64,
                                 sc["qh"] * 1024 + j * 512:
                                 sc["qh"] * 1024 + (j + 1) * 512],
                    start=True, stop=True,
                )
            sc["e_q"][hl].append(p)

    def emit_exp(sc):
        for hl in range(2):
            p = sc["e_q"][hl].pop(0)
            at = attp.tile([128, 1024], BF16, tag="att", name="at")
            nc.scalar.activation(at[:], p[:], EXPF)
            sc["a_q"][hl].append(at)

    def emit_attv(sc, kt):
        # att@V col-tiled: head h -> psum partitions 0:64 (array cols 0:64),
        # h' -> 64:128; the two matmuls stream different moving operands into
        # different column groups concurrently.  Denominators are 4 P=1
        # col-tiled matmuls (partitions 0/32/64/96 = h-j0/h-j1/h'-j0/h'-j1).
        ats = [sc["a_q"][hl].pop(0) for hl in range(2)]
        h0 = 2 * sc["p"]
        for j in range(2):
            o = sc["o"][j]
            for hl in range(2):
                nc.tensor.matmul(
                    o[hl * 64:(hl + 1) * 64, :],
                    v_t[kt][:, h0 + hl, 0:64],
                    ats[hl][:, j * 512:(j + 1) * 512],
                    start=(kt == 0), stop=(kt == NT_N - 1),
                )
        for hl in range(2):
            for j in range(2):
                bp = hl * 64 + j * 32
                nc.tensor.matmul(
                    sc["dn"][bp:bp + 1, :],
                    ones_t[:, 0:1],
                    ats[hl][:, j * 512:(j + 1) * 512],
                    start=(kt == 0), stop=(kt == NT_N - 1),
                    tile_position=(0, bp),
                )

    def drain_seg(sc):
        p_, qh = sc["p"], sc["qh"]
        # den psum rows -> sbuf -> DRAM bounce (row r = h*4 + qg)
        st = stage.tile([128, 512], F32, tag="st", name="st")
        nc.vector.tensor_copy(st[:], sc["dn"][:])
        for hl in range(2):
            for j in range(2):
                r = (2 * p_ + hl) * 4 + qh * 2 + j
                bp = hl * 64 + j * 32
                nc.gpsimd.dma_start(den_dram[r:r + 1, :], st[bp:bp + 1, :])
        # raw out^T -> sbuf bf16, both heads in one cast per q-group
        for j in range(2):
            qg = qh * 2 + j
            nc.vector.tensor_copy(
                outT_raw[p_][:, qg * 512:(qg + 1) * 512], sc["o"][j][:])
        if p_ == 3 and qh == 1:
            den_recip(32)
            for qg in (2, 3):
                norm_pair(3, qg)
        else:
            for _ in state["dve"]:
                pass
            state["dve"] = norm_gen_half(p_, qh)

    LAG = 2
    tail = []
    for s in range(HPC):
        p_, qh = s >> 1, s & 1
        if qh == 0:
            for _ in filler:  # MUST finish pair p_'s qkT before its energies
                pass
            if p_ == 0:
                filler = chain2(v_tail_gen(range(NT_N - 4, NT_N)),
                                qk_pair_gen(1))
            elif p_ + 1 < 4:
                filler = qk_pair_gen(p_ + 1)
        sc = seg_ctx(p_, qh)
        for kt in range(LAG):
            if kt >= 1:
                emit_exp(sc)
            emit_E(sc, kt)
            if tail:
                tail.pop(0)()
            for _ in range(4 if p_ == 0 else 2):
                next(filler, None)
        sc["o"] = (po.tile([128, 512], F32, tag="po", name="o0"),
                   po.tile([128, 512], F32, tag="po", name="o1"))
        sc["dn"] = dn.tile([128, 512], F32, tag="dn", name="dnp")
        for kt in range(LAG, NT_N):
            emit_exp(sc)
            emit_E(sc, kt)
            emit_attv(sc, kt - LAG)
            for _ in range(4 if p_ == 0 else 2):
                next(filler, None)
            if qh == 1:
                next(filler, None)
            if kt % 3 == 2 or kt == NT_N - 1:
                next(state["dve"], None)
        emit_exp(sc)
        tail = [(lambda sc=sc: emit_attv(sc, NT_N - 2)),
                (lambda sc=sc: (emit_attv(sc, NT_N - 1), drain_seg(sc)))]
    for fn in tail:
        fn()
    for _ in filler:
        pass
    for _ in state["dve"]:
        pass

    # ---- phase 3: partial proj  y = outT_n^T @ wp  (bf16 out) ----
    for nt in range(NT_N):
        ys = yp.tile([128, 1024], BF16, tag="y", name="ys")
        for ng in range(2):
            p = po.tile([128, 512], F32, tag="po", name="pp")
            for t4 in range(4):
                nc.tensor.matmul(
                    p[:],
                    outT_n[t4][:, nt * 128:(nt + 1) * 128],
                    wp_t[t4][:, ng * 512:(ng + 1) * 512],
                    start=(t4 == 0), stop=(t4 == 3),
                )
            if ng == 0:
                nc.vector.tensor_copy(ys[:, 0:512], p[:])
            else:
                nc.scalar.copy(ys[:, 512:1024], p[:])
        nc.sync.dma_start(y[nt * 128:(nt + 1) * 128, :], ys[:])
    if dbg is not None:
        for t in range(HPC):
            nc.sync.dma_start(dbg["qkT"][t * 128:(t + 1) * 128, :], qkT[t][:])
        nc.sync.dma_start(dbg["rec"][:, :], rec_pk[:])
        for nt in range(NT_N):
            nc.sync.dma_start(
                dbg["v"][nt * 128:(nt + 1) * 128, :, :], v_t[nt][:])
        nc.sync.dma_start(dbg["den"][:, :], den_pk[:])
        for t in range(4):
            nc.sync.dma_start(dbg["oraw"][t * 128:(t + 1) * 128, :], outT_raw[t][:])


_CACHE = {}


def _get_nc():
    if "nc" not in _CACHE:
        nc = bacc.Bacc("TRN2", target_bir_lowering=False, debug=False, num_devices=8)
        xT = nc.dram_tensor("xT", [KAUG, N], BF16, kind="ExternalInput")
        wqk = nc.dram_tensor("wqk", [KAUG, 1024], BF16, kind="ExternalInput")
        wv = nc.dram_tensor("wv", [KAUG, 512], BF16, kind="ExternalInput")
        wp = nc.dram_tensor("wp", [512, 1024], BF16, kind="ExternalInput")
        y = nc.dram_tensor("y", [N, EMB], BF16, kind="ExternalOutput")
        import os
        dbg = None
        if os.environ.get("MHA_DEBUG"):
            dbg = {
                "qkT": nc.dram_tensor("dbg_qkT", [HPC * 128, N], BF16, kind="ExternalOutput").ap(),
                "rec": nc.dram_tensor("dbg_rec", [32, 512], F32, kind="ExternalOutput").ap(),
                "den": nc.dram_tensor("dbg_den", [32, 512], F32, kind="ExternalOutput").ap(),
                "oraw": nc.dram_tensor("dbg_oraw", [512, N], BF16, kind="ExternalOutput").ap(),
                "v": nc.dram_tensor("dbg_v", [N, HPC, 65], BF16, kind="ExternalOutput").ap(),
            }
        with tile.TileContext(nc) as tc:
            from contextlib import ExitStack
            with ExitStack() as es:
                _build_kernel(es, tc, nc, xT.ap(), wqk.ap(), wv.ap(), wp.ap(), y.ap(), dbg=dbg)
        nc.compile()
        _CACHE["nc"] = nc
    return _CACHE["nc"]


def make_in_maps(x, w_qkv, b_qkv, w_proj):
    """Host-side shard prep: per-core bf16 operands with folded biases/scale."""
    x = np.asarray(x, np.float32)
    w_qkv = np.asarray(w_qkv, np.float32)
    b_qkv = np.asarray(b_qkv, np.float32)
    w_proj = np.asarray(w_proj, np.float32)
    scale = 1.0 / np.sqrt(HD)

    in_maps = []
    for c in range(8):
        b, g = divmod(c, 2)
        heads = range(g * HPC, (g + 1) * HPC)

        xT_aug = np.zeros((KAUG, N), np.float32)
        xT_aug[0:EMB, :] = x[b].T
        xT_aug[EMB, :] = 1.0

        wqk = np.zeros((KAUG, 1024), np.float32)
        wv = np.zeros((KAUG, 512), np.float32)
        for hl, H in enumerate(heads):
            qs, ks, vs = H * HD, EMB + H * HD, 2 * EMB + H * HD
            # q cols: m-tile hl//2, partition half hl%2; k cols: m-tile 4+hl//2
            qc = (hl // 2) * 128 + (hl % 2) * 64
            kc = 512 + qc
            wqk[0:EMB, qc:qc + 64] = w_qkv[qs:qs + HD, :].T * scale
            wqk[EMB, qc:qc + 64] = b_qkv[qs:qs + HD] * scale
            wqk[0:EMB, kc:kc + 64] = w_qkv[ks:ks + HD, :].T
            wqk[EMB, kc:kc + 64] = b_qkv[ks:ks + HD]
            wv[0:EMB, hl * 64:(hl + 1) * 64] = w_qkv[vs:vs + HD, :].T
            wv[EMB, hl * 64:(hl + 1) * 64] = b_qkv[vs:vs + HD]

        wp = w_proj[:, g * 512:(g + 1) * 512].T.copy()

        in_maps.append({
            "xT": xT_aug.astype(bf16),
            "wqk": wqk.astype(bf16),
            "wv": wv.astype(bf16),
            "wp": wp.astype(bf16),
        })
    return in_maps


def kernel(x, w_qkv, b_qkv, w_proj, b_proj):
    x = np.asarray(x, np.float32)
    b_proj = np.asarray(b_proj, np.float32)
    nc = _get_nc()
    in_maps = make_in_maps(x, w_qkv, b_qkv, w_proj)
    res = run_bass_kernel_spmd(nc, in_maps, core_ids=list(range(8)))
    out = np.empty((B, N, EMB), np.float32)
    for b in range(B):
        out[b] = (res.results[2 * b]["y"].astype(np.float32)
                  + res.results[2 * b + 1]["y"].astype(np.float32) + b_proj)
    return out


# revision 26
# speedup vs baseline: 1.0088x; 1.0088x over previous
"""Self-contained 8-core Trainium2 Bass kernel for nn_MultiHeadAttention.

Problem: x:[4,2048,1024] f32, w_qkv:[3072,1024], b_qkv:[3072],
w_proj:[1024,1024], b_proj:[1024].  16 heads, head_dim 64.

Sharding: core c = batch(4) x head-group(2).  Each core computes QKV for
its 8 heads on its batch, attention, and a partial output projection over
its 512 head-dims.  Host sums the two partials per batch and adds b_proj.

Per-core dataflow (all matmuls bf16, fp32 PSUM):
  - host supplies xT_aug [1152,2048] = [x_b^T; ones; 0pad] (contraction dim
    carries the bias via an augmented row), wqk [1152,1024] with column
    layout head h -> cols h*128..h*128+64 = q (pre-scaled 1/8), +64..+128 = k,
    wv [1152,512], wp [512,1024].
  - prologue: v tiles (natural [n, head, d] layout), then qk for head-pair 0;
    qk for pairs 1-3 interleave into attention PE slack as fillers.
  - qkT[h] tiles [128,2048]: partitions 0:64 = q^T of even head, 64:128 =
    odd head (d on partitions) -> energy^T = k @ q^T lands in [k,q] layout,
    so the exp output feeds att@V directly (no PE transposes anywhere).
  - attention runs per head-PAIR with PE array tiling:
      * energies: the two K=64 matmuls row-tile (array rows 0:64 / 64:128)
        and run concurrently;
      * att@V: col-tiled, head h -> psum partitions 0:64, h' -> 64:128,
        two moving streams into different column groups concurrently;
      * softmax denominators: 4 P=1 col-tiled matmuls (ones vector lhsT)
        accumulating into one den psum bank at partitions 0/32/64/96.
    Per 2.23us pair-slot (2 exps) the PE does ~1.7us of work, so the
    scalar engine's exp stream (256 x [128,1024] exp, ~1.1us each) is the
    pacing floor; emission order exp(kt-1), E(kt), attv(kt-2) keeps it fed,
    and each segment's last two attvs carry into the next segment's first
    two slots so the stream continues across boundaries.
  - energies are bounded (~|2|) so exp needs no max subtraction.
  - denominators: den psum -> sbuf -> DRAM bounce -> packed [32,512] ->
    lane-parallel DVE reciprocal -> stream_shuffle broadcast -> one
    scalar_tensor_tensor multiply normalizes out^T (interleaved as dve
    fillers into the next pair's slots).
  - y partials returned in bf16 (halves the output DMA); host sums in f32.
"""
import sys

sys.path.insert(0, "/opt/trn_rl_repo")

import numpy as np
import ml_dtypes

import concourse.bass as bass
import concourse.mybir as mybir
import concourse.tile as tile
from concourse import bacc
from concourse.bass_utils import run_bass_kernel_spmd

bf16 = ml_dtypes.bfloat16
F32 = mybir.dt.float32
BF16 = mybir.dt.bfloat16

B, N, EMB = 4, 2048, 1024
HEADS, HD = 16, 64
HPC = 8            # heads per core
KAUG = 1152        # 1024 emb + 1 bias row, padded to 9*128
NKT = KAUG // 128  # 9 contraction tiles for qkv
NT_N = N // 128    # 16 n-tiles
EXPF = mybir.ActivationFunctionType.Exp


def _build_kernel(ctx, tc, nc, xT, wqk, wv, wp, y, dbg=None):
    mult = mybir.AluOpType.mult

    const = ctx.enter_context(tc.tile_pool(name="const", bufs=1))
    qkp = ctx.enter_context(tc.tile_pool(name="qkp", bufs=1))
    vp = ctx.enter_context(tc.tile_pool(name="vp", bufs=1))
    outp = ctx.enter_context(tc.tile_pool(name="outp", bufs=1))
    attp = ctx.enter_context(tc.tile_pool(name="attp", bufs=7))
    misc = ctx.enter_context(tc.tile_pool(name="misc", bufs=1))
    stage = ctx.enter_context(tc.tile_pool(name="stage", bufs=2))
    bcp = ctx.enter_context(tc.tile_pool(name="bcp", bufs=3))
    yp = ctx.enter_context(tc.tile_pool(name="yp", bufs=4))
    pe = ctx.enter_context(tc.tile_pool(name="pe", bufs=2, space="PSUM"))
    fill = ctx.enter_context(tc.tile_pool(name="fill", bufs=1, space="PSUM"))
    po = ctx.enter_context(tc.tile_pool(name="po", bufs=2, space="PSUM"))
    dn = ctx.enter_context(tc.tile_pool(name="dn", bufs=1, space="PSUM"))
    dramp = ctx.enter_context(tc.tile_pool(name="dramp", bufs=1, space="DRAM"))

    # ---- load inputs: xT+wv first (v-phase deps), wqk next, wp last;
    # spread across three DMA queues so the prologue isn't one-queue bound ----
    xT_t = []
    wqk_t = []
    wv_t = []
    for kt in range(NKT):
        t = const.tile([128, N], BF16, tag=f"xT{kt}", name=f"xT{kt}")
        (nc.sync if kt % 2 == 0 else nc.scalar).dma_start(
            t[:], xT[kt * 128:(kt + 1) * 128, :])
        xT_t.append(t)
        t = const.tile([128, 512], BF16, tag=f"wv{kt}", name=f"wv{kt}")
        nc.gpsimd.dma_start(t[:], wv[kt * 128:(kt + 1) * 128, :])
        wv_t.append(t)
    for kt in range(NKT):
        t = const.tile([128, 1024], BF16, tag=f"wqk{kt}", name=f"wqk{kt}")
        nc.gpsimd.dma_start(t[:], wqk[kt * 128:(kt + 1) * 128, :])
        wqk_t.append(t)
    wp_t = []
    for t4 in range(4):
        t = const.tile([128, 1024], BF16, tag=f"wp{t4}", name=f"wp{t4}")
        nc.sync.dma_start(t[:], wp[t4 * 128:(t4 + 1) * 128, :])
        wp_t.append(t)

    # qk bias vectors: aug row 1024 of wqk, one [128,1] per m-tile
    bq_t = []
    for t8 in range(8):
        tb = misc.tile([128, 1], BF16, tag=f"bqb{t8}", name=f"bqb{t8}")
        nc.scalar.dma_start(tb[:], wqk[EMB:EMB + 1, t8 * 128:(t8 + 1) * 128])
        t = misc.tile([128, 1], F32, tag=f"bq{t8}", name=f"bq{t8}")
        nc.vector.tensor_copy(t[:], tb[:])
        bq_t.append(t)

    # qk m-tiles 0..3 = q of head pairs (h%2 on partition halves), 4..7 = k.
    qkT = [qkp.tile([128, N], BF16, tag=f"qkT{h}", name=f"qkT{h}") for h in range(HPC)]
    v_t = [vp.tile([128, HPC, 65], BF16, tag=f"v{nt}", name=f"v{nt}") for nt in range(NT_N)]
    outT_raw = [outp.tile([128, N], BF16, tag=f"or{t}", name=f"or{t}") for t in range(4)]
    outT_n = [outp.tile([128, N], BF16, tag=f"on{t}", name=f"on{t}") for t in range(4)]
    den_dram = dramp.tile([32, 512], F32, name="den_dram")
    den_pk = misc.tile([32, 512], F32, tag="den", name="den_pk")
    rec_pk = misc.tile([32, 512], F32, tag="rec", name="rec_pk")
    nc.vector.memset(den_pk[:], 1.0)
    ones_t = misc.tile([128, 1], BF16, tag="ones", name="ones_t")
    nc.vector.memset(ones_t[:], 1.0)

    def emit_v_tile(nt):
        p = pe.tile([128, 1024], F32, tag="pe", name="pep")
        for kt in range(NKT):
            nc.tensor.matmul(
                p[:, 0:512],
                xT_t[kt][:, nt * 128:(nt + 1) * 128],
                wv_t[kt][:],
                start=(kt == 0), stop=(kt == NKT - 1),
            )
        nc.scalar.copy(
            v_t[nt][:, :, 0:64],
            p[:, 0:512].rearrange("p (h c) -> p h c", c=64),
        )
        nc.vector.memset(v_t[nt][:, :, 64:65], 1.0)

    def qk_pair_gen(hp, prologue=False):
        """Generator emitting one PE matmul per step for q/k m-tiles of pair hp.

        prologue=True (pair 0): full [128,1024] psum groups from the pe pool
        (double-buffered, fastest serial path).  Fillers use a single-bank
        [128,512] psum so po can triple-buffer: each j-half gets its own
        accumulation group and bias-add."""
        for t in (hp, 4 + hp):
            for nbp in range(2):
                if prologue:
                    p = pe.tile([128, 1024], F32, tag="pe", name="pep")
                    for j in range(2):
                        pj = p[:, j * 512:(j + 1) * 512]
                        for kt in range(NKT - 1):
                            nc.tensor.matmul(
                                pj,
                                wqk_t[kt][:, t * 128:(t + 1) * 128],
                                xT_t[kt][:, nbp * 1024 + j * 512:
                                         nbp * 1024 + (j + 1) * 512],
                                start=(kt == 0), stop=(kt == NKT - 2),
                            )
                            if not (kt == NKT - 2 and j == 1):
                                yield
                    nc.vector.tensor_scalar_add(
                        qkT[t][:, nbp * 1024:(nbp + 1) * 1024], p[:], bq_t[t][:])
                    yield
                else:
                    for j in range(2):
                        p = fill.tile([128, 512], F32, tag="fl", name="flp")
                        c = nbp * 1024 + j * 512
                        for kt in range(NKT - 1):
                            nc.tensor.matmul(
                                p[:],
                                wqk_t[kt][:, t * 128:(t + 1) * 128],
                                xT_t[kt][:, c:c + 512],
                                start=(kt == 0), stop=(kt == NKT - 2),
                            )
                            yield
                        nc.vector.tensor_scalar_add(
                            qkT[t][:, c:c + 512], p[:], bq_t[t][:])
                        yield
                        yield
                        yield

    def norm_pair(p_, qg):
        # both heads of the pair in one STT: bc rows 0:64 = 1/den of head
        # 2p, 64:128 = head 2p+1 (partition-parallel, so the [128,512] STT
        # costs the same as a [64,512] one)
        bc = bcp.tile([128, 512], F32, tag="bc", name="bc")
        m0 = [(2 * p_) * 4 + qg] * 32
        m1 = [(2 * p_ + 1) * 4 + qg] * 32
        nc.vector.stream_shuffle(bc[0:32, :], rec_pk[0:32, :], mask=m0)
        nc.vector.stream_shuffle(bc[32:64, :], rec_pk[0:32, :], mask=m0)
        nc.vector.stream_shuffle(bc[64:96, :], rec_pk[0:32, :], mask=m1)
        nc.vector.stream_shuffle(bc[96:128, :], rec_pk[0:32, :], mask=m1)
        nc.vector.scalar_tensor_tensor(
            outT_n[p_][:, qg * 512:(qg + 1) * 512],
            outT_raw[p_][:, qg * 512:(qg + 1) * 512],
            1.0,
            bc[:, :],
            op0=mult, op1=mult,
        )

    def norm_gen_half(p_, qh):
        # qh==0: normalize q-groups 0,1 of pair p (their outT_raw/den rows
        # drained at mid-pair); qh==1: q-groups 2,3 (drained at pair end).
        # The den reload+reciprocal is split into two half-width steps so no
        # single DVE op exceeds ~1.7us -- keeps the filler bias-adds (which
        # the single-buffered fill psum WARs on) from queueing behind it.
        nrows = 8 * p_ + 6 if qh == 0 else 8 * (p_ + 1)
        nc.gpsimd.dma_start(den_pk[0:nrows, :], den_dram[0:nrows, :])
        nc.vector.reciprocal(rec_pk[:, 0:256], den_pk[:, 0:256])
        yield
        nc.vector.reciprocal(rec_pk[:, 256:512], den_pk[:, 256:512])
        yield
        for qg in (2 * qh, 2 * qh + 1):
            norm_pair(p_, qg)
            yield
            yield

    def den_recip(nrows):
        nc.gpsimd.dma_start(den_pk[0:nrows, :], den_dram[0:nrows, :])
        nc.vector.reciprocal(rec_pk[:], den_pk[:])

    # ---- prologue: v tiles, then qk for head-pair 0 (PE-only, ACT idle) ----
    for nt in range(NT_N):
        emit_v_tile(nt)
    for _ in qk_pair_gen(0, prologue=True):
        pass

    # ---- attention.  The scalar engine's exp stream (one [128,1024] exp per
    # kt, ~1us each, 256 total) is the throughput floor.  Per-slot emission
    # order is exp(kt-1), energy(kt), attv(kt-2): the next energy issues on
    # the PE right behind the previous exp so ACT never waits.  The two
    # trailing attvs of each (h,qh) segment carry into the next segment's
    # first two slots so the exp stream continues across boundaries.  qk
    # matmuls for the next head pair interleave as fillers into PE slack;
    # each pair's generator MUST be fully drained before the pair's energies
    # are emitted (the drain at each even h) -- the energies read qkT tiles
    # that the filler writes. ----
    filler = iter(())
    state = {"dve": iter(())}

    def seg_ctx(p_, qh):
        return {"p": p_, "qh": qh, "e_q": [[], []], "a_q": [[], []],
                "o": None, "dn": None}

    def emit_E(sc, kt):
        # energies for both heads of the pair, row-tiled (h: array rows 0:64,
        # h': rows 64:128) so the two K=64 matmuls run concurrently.
        for hl in range(2):
            pb = hl * 64
            p = pe.tile([128, 1024], F32, tag="pe", name="pep")
            for j in range(2):
                nc.tensor.matmul(
                    p[:, j * 512:(j + 1) * 512],
                    qkT[4 + sc["p"]][pb:pb + 64, kt * 128:(kt + 1) * 128],
                    qkT[sc["p"]][pb:pb + 64,
                                 sc["qh"] * 1024 + j * 512:
                                 sc["qh"] * 1024 + (j + 1) * 512],
                    start=True, stop=True,
                )
            sc["e_q"][hl].append(p)

    def emit_exp(sc):
        for hl in range(2):
            p = sc["e_q"][hl].pop(0)
            at = attp.tile([128, 1024], BF16, tag="att", name="at")
            nc.scalar.activation(at[:], p[:], EXPF)
            sc["a_q"][hl].append(at)

    def emit_attv(sc, kt):
        # att@V col-tiled: head h -> psum partitions 0:64 (array cols 0:64),
        # h' -> 64:128; the two matmuls stream different moving operands into
        # different column groups concurrently.  Denominators are 4 P=1
        # col-tiled matmuls (partitions 0/32/64/96 = h-j0/h-j1/h'-j0/h'-j1).
        ats = [sc["a_q"][hl].pop(0) for hl in range(2)]
        h0 = 2 * sc["p"]
        for j in range(2):
            o = sc["o"][j]
            for hl in range(2):
                nc.tensor.matmul(
                    o[hl * 64:(hl + 1) * 64, :],
                    v_t[kt][:, h0 + hl, 0:64],
                    ats[hl][:, j * 512:(j + 1) * 512],
                    start=(kt == 0), stop=(kt == NT_N - 1),
                )
        for hl in range(2):
            for j in range(2):
                bp = hl * 64 + j * 32
                nc.tensor.matmul(
                    sc["dn"][bp:bp + 1, :],
                    ones_t[:, 0:1],
                    ats[hl][:, j * 512:(j + 1) * 512],
                    start=(kt == 0), stop=(kt == NT_N - 1),
                    tile_position=(0, bp),
                )

    def drain_seg(sc):
        p_, qh = sc["p"], sc["qh"]
        # den psum rows -> sbuf -> DRAM bounce (row r = h*4 + qg)
        st = stage.tile([128, 512], F32, tag="st", name="st")
        nc.vector.tensor_copy(st[:], sc["dn"][:])
        for hl in range(2):
            for j in range(2):
                r = (2 * p_ + hl) * 4 + qh * 2 + j
                bp = hl * 64 + j * 32
                nc.gpsimd.dma_start(den_dram[r:r + 1, :], st[bp:bp + 1, :])
        # raw out^T -> sbuf bf16, both heads in one cast per q-group
        for j in range(2):
            qg = qh * 2 + j
            nc.vector.tensor_copy(
                outT_raw[p_][:, qg * 512:(qg + 1) * 512], sc["o"][j][:])
        if p_ == 3 and qh == 1:
            den_recip(32)
            for qg in (2, 3):
                norm_pair(3, qg)
        else:
            for _ in state["dve"]:
                pass
            state["dve"] = norm_gen_half(p_, qh)

    LAG = 2
    tail = []
    for s in range(HPC):
        p_, qh = s >> 1, s & 1
        if qh == 0:
            for _ in filler:  # MUST finish pair p_'s qkT before its energies
                pass
            if p_ + 1 < 4:
                filler = qk_pair_gen(p_ + 1)
        sc = seg_ctx(p_, qh)
        for kt in range(LAG):
            if kt >= 1:
                emit_exp(sc)
            emit_E(sc, kt)
            if tail:
                tail.pop(0)()
            next(filler, None)
            next(filler, None)
        sc["o"] = (po.tile([128, 512], F32, tag="po", name="o0"),
                   po.tile([128, 512], F32, tag="po", name="o1"))
        sc["dn"] = dn.tile([128, 512], F32, tag="dn", name="dnp")
        for kt in range(LAG, NT_N):
            emit_exp(sc)
            emit_E(sc, kt)
            emit_attv(sc, kt - LAG)
            next(filler, None)
            next(filler, None)
            if qh == 1:
                next(filler, None)
            if kt % 3 == 2 or kt == NT_N - 1:
                next(state["dve"], None)
        emit_exp(sc)
        tail = [(lambda sc=sc: emit_attv(sc, NT_N - 2)),
                (lambda sc=sc: (emit_attv(sc, NT_N - 1), drain_seg(sc)))]
    for fn in tail:
        fn()
    for _ in filler:
        pass
    for _ in state["dve"]:
        pass

    # ---- phase 3: partial proj  y = outT_n^T @ wp  (bf16 out) ----
    for nt in range(NT_N):
        ys = yp.tile([128, 1024], BF16, tag="y", name="ys")
        for ng in range(2):
            p = po.tile([128, 512], F32, tag="po", name="pp")
            for t4 in range(4):
                nc.tensor.matmul(
                    p[:],
                    outT_n[t4][:, nt * 128:(nt + 1) * 128],
                    wp_t[t4][:, ng * 512:(ng + 1) * 512],
                    start=(t4 == 0), stop=(t4 == 3),
                )
            if ng == 0:
                nc.vector.tensor_copy(ys[:, 0:512], p[:])
            else:
                nc.scalar.copy(ys[:, 512:1024], p[:])
        nc.sync.dma_start(y[nt * 128:(nt + 1) * 128, :], ys[:])
    if dbg is not None:
        for t in range(HPC):
            nc.sync.dma_start(dbg["qkT"][t * 128:(t + 1) * 128, :], qkT[t][:])
        nc.sync.dma_start(dbg["rec"][:, :], rec_pk[:])
        for nt in range(NT_N):
            nc.sync.dma_start(
                dbg["v"][nt * 128:(nt + 1) * 128, :, :], v_t[nt][:])
        nc.sync.dma_start(dbg["den"][:, :], den_pk[:])
        for t in range(4):
            nc.sync.dma_start(dbg["oraw"][t * 128:(t + 1) * 128, :], outT_raw[t][:])


_CACHE = {}


def _get_nc():
    if "nc" not in _CACHE:
        nc = bacc.Bacc("TRN2", target_bir_lowering=False, debug=False, num_devices=8)
        xT = nc.dram_tensor("xT", [KAUG, N], BF16, kind="ExternalInput")
        wqk = nc.dram_tensor("wqk", [KAUG, 1024], BF16, kind="ExternalInput")
        wv = nc.dram_tensor("wv", [KAUG, 512], BF16, kind="ExternalInput")
        wp = nc.dram_tensor("wp", [512, 1024], BF16, kind="ExternalInput")
        y = nc.dram_tensor("y", [N, EMB], BF16, kind="ExternalOutput")
        import os
        dbg = None
        if os.environ.get("MHA_DEBUG"):
            dbg = {
                "qkT": nc.dram_tensor("dbg_qkT", [HPC * 128, N], BF16, kind="ExternalOutput").ap(),
                "rec": nc.dram_tensor("dbg_rec", [32, 512], F32, kind="ExternalOutput").ap(),
                "den": nc.dram_tensor("dbg_den", [32, 512], F32, kind="ExternalOutput").ap(),
                "oraw": nc.dram_tensor("dbg_oraw", [512, N], BF16, kind="ExternalOutput").ap(),
                "v": nc.dram_tensor("dbg_v", [N, HPC, 65], BF16, kind="ExternalOutput").ap(),
            }
        with tile.TileContext(nc) as tc:
            from contextlib import ExitStack
            with ExitStack() as es:
                _build_kernel(es, tc, nc, xT.ap(), wqk.ap(), wv.ap(), wp.ap(), y.ap(), dbg=dbg)
        nc.compile()
        _CACHE["nc"] = nc
    return _CACHE["nc"]


def make_in_maps(x, w_qkv, b_qkv, w_proj):
    """Host-side shard prep: per-core bf16 operands with folded biases/scale."""
    x = np.asarray(x, np.float32)
    w_qkv = np.asarray(w_qkv, np.float32)
    b_qkv = np.asarray(b_qkv, np.float32)
    w_proj = np.asarray(w_proj, np.float32)
    scale = 1.0 / np.sqrt(HD)

    in_maps = []
    for c in range(8):
        b, g = divmod(c, 2)
        heads = range(g * HPC, (g + 1) * HPC)

        xT_aug = np.zeros((KAUG, N), np.float32)
        xT_aug[0:EMB, :] = x[b].T
        xT_aug[EMB, :] = 1.0

        wqk = np.zeros((KAUG, 1024), np.float32)
        wv = np.zeros((KAUG, 512), np.float32)
        for hl, H in enumerate(heads):
            qs, ks, vs = H * HD, EMB + H * HD, 2 * EMB + H * HD
            # q cols: m-tile hl//2, partition half hl%2; k cols: m-tile 4+hl//2
            qc = (hl // 2) * 128 + (hl % 2) * 64
            kc = 512 + qc
            wqk[0:EMB, qc:qc + 64] = w_qkv[qs:qs + HD, :].T * scale
            wqk[EMB, qc:qc + 64] = b_qkv[qs:qs + HD] * scale
            wqk[0:EMB, kc:kc + 64] = w_qkv[ks:ks + HD, :].T
            wqk[EMB, kc:kc + 64] = b_qkv[ks:ks + HD]
            wv[0:EMB, hl * 64:(hl + 1) * 64] = w_qkv[vs:vs + HD, :].T
            wv[EMB, hl * 64:(hl + 1) * 64] = b_qkv[vs:vs + HD]

        wp = w_proj[:, g * 512:(g + 1) * 512].T.copy()

        in_maps.append({
            "xT": xT_aug.astype(bf16),
            "wqk": wqk.astype(bf16),
            "wv": wv.astype(bf16),
            "wp": wp.astype(bf16),
        })
    return in_maps


def kernel(x, w_qkv, b_qkv, w_proj, b_proj):
    x = np.asarray(x, np.float32)
    b_proj = np.asarray(b_proj, np.float32)
    nc = _get_nc()
    in_maps = make_in_maps(x, w_qkv, b_qkv, w_proj)
    res = run_bass_kernel_spmd(nc, in_maps, core_ids=list(range(8)))
    out = np.empty((B, N, EMB), np.float32)
    for b in range(B):
        out[b] = (res.results[2 * b]["y"].astype(np.float32)
                  + res.results[2 * b + 1]["y"].astype(np.float32) + b_proj)
    return out
